# revision 9
# baseline (speedup 1.0000x reference)
"""MoE layer (8 experts, top-2) on 8 TRN2 NeuronCores — expert parallelism.

Contract: kernel(**inputs) takes FULL inputs, returns FULL output.
Strategy (v2 — bf16 weights/activations, token-moving both stages):
  - Host computes the (tiny) gate: logits -> top-2 -> softmax, gathers
    tokens per expert (padded to capacity C), converts x/w1/w2 to bf16.
  - Core e runs expert e: hT = relu(w1.T @ xg + b1) block-by-block over F,
    y[d, c] += w2_tile.T @ hT — tokens are the moving dim in BOTH stages,
    so compute width is the actual max load C_ACT (<= C), not padded C.
  - y returns as [128, 8, C] (partition = d%128); host scales by the gate
    prob and scatter-adds (combine step) plus the (usually zero) b2 term.

Shapes (hardcoded from the problem spec):
  x [2048, 2, 1024], gate_w [1024, 8], gate_b [8],
  w1 [8, 1024, 4096], b1 [8, 4096], w2 [8, 4096, 1024], b2 [8, 1024].
"""
import sys
import numpy as np

for _p in ("/opt/trn_rl_repo", "/root/.axon_site/_ro/trn_rl_repo"):
    if _p not in sys.path:
        sys.path.insert(0, _p)

import concourse.bacc as bacc
import concourse.tile as tile
import concourse.mybir as mybir
from concourse import bass2jax, mybir as _mybir

N_EXPERTS = 8
TOP_K = 2
S, B, D, F = 2048, 2, 1024, 4096
P = 128
FB = 512                # F-block size streamed through SBUF
NB = F // FB            # 8 F-blocks
FC = FB // P            # 4 partition-tiles of F per block
DK = D // P             # 8 contraction tiles for stage 1
DO = D // P             # 8 output-partition tiles for stage 2

_f32 = mybir.dt.float32
_bf16 = mybir.dt.bfloat16
_np_bf16 = mybir.dt.np(_bf16)

_NC_CACHE: dict = {}
_C_MAX = 1280           # max capacity per pass (SBUF budget bound)
LAST_DEVICE_NS = -1     # wall-clock of the last device dispatch (incl. transfers)
LAST_C = -1
LAST_CACT = -1


def _c_chunks(C_act):
    """Split C_act (multiple of 4) into k=ceil(C/512) near-equal chunks.

    Chunk sizes are multiples of 4, <=512 (one PSUM bank of fp32) and
    >=256 whenever C_act >= 256 (k chunks only exist when C > 512(k-1)).
    """
    k = -(-C_act // 512)
    base = (C_act // k) // 4 * 4
    sizes = [base] * k
    rem = C_act - base * k
    i = 0
    while rem > 0:
        sizes[i] += 4
        rem -= 4
        i = (i + 1) % k
    out, pos = [], 0
    for s in sizes:
        out.append((pos, s))
        pos += s
    return out


def _build(C, passes=1, *, c_act=None, skew=True, psum_bufs=8, w_bufs=2,
           h_bufs=2, w1_bufs=None, span=2, dve_relu=True, timing_nodma=False):
    """Trace + compile the per-core SPMD program.

    C: buffer capacity (multiple of 128, >=256). c_act: computed token
    width (multiple of 4, 256 <= c_act <= C); columns >= c_act are never
    touched. passes>1 repeats the whole compute (same output) — used only
    for differential timing of the device kernel.
    span: F-blocks accumulated per stage-2 PSUM group (1 or 2); span=2
    halves the stage-2 psum recycle + DVE traffic per pass.
    timing_nodma: reuse block-0 weights for every block (wrong numerics,
    timing-only) to isolate the per-pass DMA cost.
    """
    if c_act is None:
        c_act = C
    if w1_bufs is None:
        w1_bufs = w_bufs
    if span == 2:
        w_bufs = max(w_bufs, 4)
        h_bufs = max(h_bufs, 4)
    key = (C, passes, c_act, skew, psum_bufs, w_bufs, h_bufs, w1_bufs, span,
           dve_relu, timing_nodma)
    if key in _NC_CACHE:
        return _NC_CACHE[key]
    nc = bacc.Bacc("TRN2", target_bir_lowering=False, debug=False,
                   enable_asserts=False, num_devices=8)
    # host pre-arranged layouts (see _prep_weights): partition dim first
    xg_d = nc.dram_tensor("xg", (P, DK, C), _bf16, kind="ExternalInput").ap()
    w1_d = nc.dram_tensor("w1", (P, NB, DK * FB), _bf16, kind="ExternalInput").ap()
    b1_d = nc.dram_tensor("b1", (F,), _f32, kind="ExternalInput").ap()
    w2_d = nc.dram_tensor("w2", (P, NB * FC, D), _bf16, kind="ExternalInput").ap()
    y_d = nc.dram_tensor("y", (P, DO, C), _f32, kind="ExternalOutput").ap()

    chunks = _c_chunks(c_act)

    with tile.TileContext(nc) as tc:
        with tc.tile_pool(name="const", bufs=1) as cpool, \
             tc.tile_pool(name="w1p", bufs=w1_bufs) as w1pool, \
             tc.tile_pool(name="w2p", bufs=w_bufs) as w2pool, \
             tc.tile_pool(name="hp", bufs=h_bufs) as hpool, \
             tc.tile_pool(name="ps", bufs=psum_bufs, space="PSUM") as psum:

            # Block-0 head is DMA-bound: interleave xg's dk slices with
            # w1-block0 dk slices so block-0 stage1 (emitted dk-major in
            # waves) can start as soon as the early dk slices land.
            xg_sb = cpool.tile([P, DK, C], _bf16)
            w1_first = w1pool.tile([P, DK, FB], _bf16, tag="w1_t")
            w2_first = w2pool.tile([P, FC, D], _bf16, tag="w2_t")
            for dk in range(DK):
                nc.sync.dma_start(xg_sb[:, dk, :], xg_d[:, dk, :])
                nc.sync.dma_start(w1_first[:, dk], w1_d[:, 0, dk * FB:(dk + 1) * FB])
                if 4 <= dk < 4 + FC:
                    fk = dk - 4
                    nc.sync.dma_start(w2_first[:, fk], w2_d[:, fk, :])
            b1_sb = cpool.tile([P, F // P], _f32)
            nc.sync.dma_start(b1_sb[:], b1_d.rearrange("(o p) -> p o", p=P))
            y_acc = cpool.tile([P, DO, C], _f32)

            def stage1(fb, first_block=False):
                """load w1/w2 block, produce hT = relu(w1.T @ xg + b1)."""
                if first_block or timing_nodma:
                    w1_t, w2_t = w1_first, w2_first
                else:
                    w1_t = w1pool.tile([P, DK, FB], _bf16)
                    nc.sync.dma_start(
                        w1_t[:].rearrange("p dk f -> p (dk f)"), w1_d[:, fb, :])
                    w2_t = w2pool.tile([P, FC, D], _bf16)
                    nc.sync.dma_start(
                        w2_t[:].rearrange("p fk d -> p (fk d)"),
                        w2_d[:, fb * FC:(fb + 1) * FC, :].rearrange("p fk d -> p (fk d)"))
                hT = hpool.tile([P, FC, C], _bf16)
                # chunk-major so stage2's chunk-0 groups see all 4 fc relus early
                groups = [(fc, ci) for ci in range(len(chunks)) for fc in range(FC)]

                def relu_group(fc, ci, ps, on_dve=False):
                    cs, csz = chunks[ci]
                    fcol = fb * FC + fc
                    if on_dve:
                        # relu(ps + b1) in one DVE op — block-0 only, where
                        # the serial ACT relu trail gates psum slot recycling
                        nc.vector.tensor_scalar(
                            hT[:, fc, cs:cs + csz], ps[:, :csz],
                            b1_sb[:, fcol:fcol + 1], 0.0,
                            mybir.AluOpType.add, mybir.AluOpType.max,
                        )
                    else:
                        nc.scalar.activation(
                            hT[:, fc, cs:cs + csz], ps[:, :csz],
                            mybir.ActivationFunctionType.Relu,
                            bias=b1_sb[:, fcol:fcol + 1], scale=1.0,
                        )

                if first_block:
                    # dk-major waves: up to psum_bufs groups accumulate
                    # concurrently so PE consumes each xg[dk] as it lands;
                    # chunk-major so stage2's first groups complete early
                    groups.sort(key=lambda g: (g[1], g[0]))
                    for ws in range(0, len(groups), psum_bufs):
                        wave = groups[ws:ws + psum_bufs]
                        pss = [psum.tile([P, 512], _f32, name=f"ps_w{ws}_{i}", tag="ps")
                               for i in range(len(wave))]
                        for dk in range(DK):
                            for (fc, ci), ps in zip(wave, pss):
                                cs, csz = chunks[ci]
                                nc.tensor.matmul(
                                    ps[:, :csz],
                                    w1_t[:, dk, fc * P:(fc + 1) * P],
                                    xg_sb[:, dk, cs:cs + csz],
                                    start=(dk == 0), stop=(dk == DK - 1),
                                )
                        for i, ((fc, ci), ps) in enumerate(zip(wave, pss)):
                            relu_group(fc, ci, ps, on_dve=(i % 2 == 0))
                else:
                    for gi, (fc, ci) in enumerate(groups):
                        cs, csz = chunks[ci]
                        ps = psum.tile([P, 512], _f32, tag="ps")
                        for dk in range(DK):
                            nc.tensor.matmul(
                                ps[:, :csz],
                                w1_t[:, dk, fc * P:(fc + 1) * P],
                                xg_sb[:, dk, cs:cs + csz],
                                start=(dk == 0), stop=(dk == DK - 1),
                            )
                        relu_group(fc, ci, ps, on_dve=(dve_relu and gi % 2 == 0))
                return hT, w2_t

            def stage2(pair, first, last):
                """y_acc[d%128, d//128, c] (+)= w2_tile.T @ hT over the pair's
                F-blocks (one PSUM accumulation group each); store on last."""
                nmm = len(pair) * FC
                for do in range(DO):
                    for ci, (cs, csz) in enumerate(chunks):
                        ps2 = psum.tile([P, 512], _f32, tag="ps")
                        mi = 0
                        for hT, w2_t in pair:
                            for fk in range(FC):
                                nc.tensor.matmul(
                                    ps2[:, :csz],
                                    w2_t[:, fk, do * P:(do + 1) * P],
                                    hT[:, fk, cs:cs + csz],
                                    start=(mi == 0), stop=(mi == nmm - 1),
                                )
                                mi += 1
                        ya = y_acc[:, do, cs:cs + csz]
                        if first:
                            nc.vector.tensor_copy(ya, ps2[:, :csz])
                        else:
                            nc.vector.tensor_add(ya, ps2[:, :csz], ya)
                    if last:
                        nc.sync.dma_start(y_d[:, do, :c_act], y_acc[:, do, :c_act])

            n_blocks = passes * NB
            tiles = {}

            def emit_s1(rep):
                tiles[rep] = stage1(rep % NB, first_block=(rep == 0))

            def emit_s2(rep0, nrep):
                pair = [tiles.pop(rep0 + i) for i in range(nrep)]
                fb_last = (rep0 + nrep - 1) % NB
                stage2(pair, rep0 % NB == 0, fb_last == NB - 1)

            if span == 2:
                # pairs (2k, 2k+1); s2(pair k-1) emitted between the next
                # pair's two stage1s so the relu trail hides under stage1.
                emit_s1(0)
                emit_s1(1)
                for k in range(1, n_blocks // 2):
                    emit_s1(2 * k)
                    emit_s2(2 * (k - 1), 2)
                    emit_s1(2 * k + 1)
                emit_s2(n_blocks - 2, 2)
            elif skew and n_blocks >= 2:
                # Block 0 is DMA-bound: emit s2(0) right after s1(0) (PE is
                # waiting on DMA there anyway, the relu bubble is free), then
                # skew by one block so later relus hide under the next s1.
                emit_s1(0)
                emit_s2(0, 1)
                emit_s1(1)
                for rep in range(2, n_blocks):
                    emit_s1(rep)
                    emit_s2(rep - 1, 1)
                emit_s2(n_blocks - 1, 1)
            else:
                for rep in range(n_blocks):
                    emit_s1(rep)
                    emit_s2(rep, 1)
    nc.compile()
    _NC_CACHE[key] = nc
    return nc


class _Runner:
    """Persistent jitted SPMD executor for a compiled Bacc program.

    Mirrors bass2jax.run_bass_via_pjrt but keeps the jitted callable so
    repeat calls skip retracing/recompiling.
    """

    def __init__(self, nc, n_cores):
        import jax
        from jax.sharding import Mesh, PartitionSpec
        from jax.experimental.shard_map import shard_map

        bass2jax.install_neuronx_cc_hook()
        self.nc = nc
        self.n_cores = n_cores
        in_names, out_names, out_avals = [], [], []
        for alloc in nc.m.functions[0].allocations:
            if not isinstance(alloc, _mybir.MemoryLocationSet):
                continue
            name = alloc.memorylocations[0].name
            if alloc.kind == "ExternalInput":
                in_names.append(name)
            elif alloc.kind == "ExternalOutput":
                out_names.append(name)
                out_avals.append(jax.core.ShapedArray(
                    tuple(alloc.tensor_shape), _mybir.dt.np(alloc.dtype)))
        partition_name = nc.partition_id_tensor.name if nc.partition_id_tensor else None
        in_names = [n for n in in_names if n != partition_name]
        all_names = in_names + out_names + ([partition_name] if partition_name else [])
        self.in_names, self.out_names, self.out_avals = in_names, out_names, out_avals
        self._all_names, self._partition_name = all_names, partition_name
        n_params = len(in_names)

        def _body(*args):
            operands = list(args)
            if partition_name is not None:
                operands.append(bass2jax.partition_id_tensor())
            outs = bass2jax._bass_exec_p.bind(
                *operands,
                out_avals=tuple(out_avals),
                in_names=tuple(all_names),
                out_names=tuple(out_names),
                lowering_input_output_aliases=(),
                sim_require_finite=False,
                sim_require_nnan=False,
                nc=nc,
            )
            return tuple(outs)

        devices = jax.devices()[:n_cores]
        mesh = Mesh(np.asarray(devices), ("core",))
        n_outs = len(out_names)
        self._fn = jax.jit(
            shard_map(_body, mesh=mesh,
                      in_specs=(PartitionSpec("core"),) * (n_params + n_outs),
                      out_specs=(PartitionSpec("core"),) * n_outs,
                      check_rep=False),
            donate_argnums=tuple(range(n_params, n_params + n_outs)),
            keep_unused=True,
        )
        self._jax = jax

    def concat_inputs(self, in_maps):
        return [np.concatenate([np.asarray(m[name]) for m in in_maps], axis=0)
                for name in self.in_names]

    def zero_outs(self):
        jnp = self._jax.numpy
        return [jnp.zeros((self.n_cores * a.shape[0], *a.shape[1:]), a.dtype)
                for a in self.out_avals]

    def run_raw(self, concat_in, zouts):
        outs = self._fn(*concat_in, *zouts)
        self._jax.block_until_ready(outs)
        return outs

    def run(self, in_maps):
        outs = self.run_raw(self.concat_inputs(in_maps), self.zero_outs())
        return [
            {name: np.asarray(outs[i]).reshape(self.n_cores, *self.out_avals[i].shape)[c]
             for i, name in enumerate(self.out_names)}
            for c in range(self.n_cores)
        ]


_RUNNER_CACHE: dict = {}


def _runner(C, passes=1, c_act=None):
    key = (C, passes, c_act)
    if key not in _RUNNER_CACHE:
        _RUNNER_CACHE[key] = _Runner(_build(C, passes, c_act=c_act), N_EXPERTS)
    return _RUNNER_CACHE[key]


def _route(x2d, gate_w, gate_b):
    """Host gate: returns per-token top-2 expert ids and softmax probs (fp32)."""
    logits = x2d.astype(np.float64) @ gate_w.astype(np.float64) + gate_b.astype(np.float64)
    order = np.argsort(-logits, axis=-1, kind="stable")
    top2 = order[:, :TOP_K]                               # [T, 2]
    l = np.take_along_axis(logits, top2, axis=-1)         # [T, 2]
    m = l.max(axis=-1, keepdims=True)
    e = np.exp(l - m)
    p = (e / e.sum(axis=-1, keepdims=True)).astype(np.float32)
    return top2, p


def _prep_weights(w1_e, w2_e):
    """Pre-arrange one expert's weights into the device DMA layouts (bf16).

    w1 [D, F] -> [128(ki), NB, DK*FB] with inner order (ko, fcol):
      block fb's slice [:, fb, :] is one contiguous 8KB/partition DMA.
    w2 [F, D] -> [128(fi), NB*FC, D]: block fb's [:, 4fb:4fb+4, :] ditto.
    """
    w1p = w1_e.reshape(DK, P, NB, FB).transpose(1, 2, 0, 3)      # ki, fb, ko, fcol
    w1p = np.ascontiguousarray(w1p, dtype=_np_bf16).reshape(P, NB, DK * FB)
    w2p = w2_e.reshape(NB * FC, P, D).transpose(1, 0, 2)          # fi, fo, d
    w2p = np.ascontiguousarray(w2p, dtype=_np_bf16)
    return w1p, w2p


def kernel(x, gate_w, gate_b, w1, b1, w2, b2):
    x = np.asarray(x, dtype=np.float32)
    gate_w = np.asarray(gate_w, dtype=np.float32)
    gate_b = np.asarray(gate_b, dtype=np.float32)
    w1 = np.asarray(w1, dtype=np.float32)
    b1 = np.asarray(b1, dtype=np.float32)
    w2 = np.asarray(w2, dtype=np.float32)
    b2 = np.asarray(b2, dtype=np.float32)

    T = S * B
    x2d = np.ascontiguousarray(x.reshape(T, D))
    top2, p = _route(x2d, gate_w, gate_b)

    # dispatch: token lists per expert
    idx_lists = []
    for e in range(N_EXPERTS):
        sel = np.nonzero(top2 == e)          # (token_idx, slot_idx)
        idx_lists.append((sel[0], p[sel[0], sel[1]]))
    max_n = max(len(ix) for ix, _ in idx_lists)

    # capacity cap (SBUF budget): if wildly imbalanced, run multiple passes
    n_pass = max(1, -(-max_n // _C_MAX))
    per_pass = -(-max_n // n_pass)
    C = max(256, -(-per_pass // P) * P)
    c_act = max(256, -(-min(per_pass, C) // 4) * 4)

    global LAST_C, LAST_CACT
    LAST_C = C
    LAST_CACT = c_act
    runner = _runner(C, c_act=c_act)

    out2d = np.zeros((T, D), dtype=np.float32)
    xT = x2d.T  # [D, T]
    wprep = [_prep_weights(w1[e], w2[e]) for e in range(N_EXPERTS)]
    for ps in range(n_pass):
        in_maps = []
        metas = []
        for e in range(N_EXPERTS):
            ix_all, pe_all = idx_lists[e]
            ix = ix_all[ps * C:(ps + 1) * C]
            pe = pe_all[ps * C:(ps + 1) * C]
            n = len(ix)
            xg = np.zeros((P, DK, C), dtype=_np_bf16)
            if n:
                # [D, n] -> [128(ki), DK(ko), n]
                xg[:, :, :n] = xT[:, ix].astype(_np_bf16).reshape(DK, P, n).transpose(1, 0, 2)
            in_maps.append({
                "xg": xg,
                "w1": wprep[e][0],
                "b1": np.ascontiguousarray(b1[e]),
                "w2": wprep[e][1],
            })
            metas.append((ix, pe, n))
        import time as _time
        _t0 = _time.time()
        results = runner.run(in_maps)
        global LAST_DEVICE_NS
        LAST_DEVICE_NS = int((_time.time() - _t0) * 1e9)
        for e in range(N_EXPERTS):
            ix, pe, n = metas[e]
            if n:
                y = results[e]["y"]                  # [128(di), DO, C]
                y2 = y.transpose(1, 0, 2).reshape(D, C)[:, :n]   # [D, n]
                out2d[ix] += (y2 * pe).T             # ix unique per expert

    if np.any(b2):
        comb = np.zeros((T, N_EXPERTS), dtype=np.float32)
        np.put_along_axis(comb, top2, p, axis=-1)
        out2d += comb @ b2
    return out2d.reshape(S, B, D)


# revision 14
# speedup vs baseline: 1.0824x; 1.0824x over previous
"""MoE layer (8 experts, top-2) on 8 TRN2 NeuronCores — expert parallelism.

Contract: kernel(**inputs) takes FULL inputs, returns FULL output.
Strategy (v3 — bf16 weights/activations, token-moving, slot-balanced):
  - Host computes the (tiny) gate: logits -> top-2 -> softmax, gathers
    tokens per expert, converts x/w1/w2 to bf16.
  - Each core's token columns are split into 1-2 slots, each slot bound
    to one expert's weights; the host picks slot sizes/assignment so the
    per-core width (the SPMD compute cost) is ~the mean load, not the max
    (two biggest experts split across two A slots, two smallest share B
    slots, middle four take one A + one B).
  - Per slot: hT = relu(w1.T @ xg + b1) block-by-block over F, then
    y[d, c] += w2_tile.T @ hT — tokens are the moving dim in BOTH stages.
  - y returns as [128, 8, Ctot] (partition = d%128); host scales by the
    gate prob and scatter-adds (combine) plus the (usually zero) b2 term.

Shapes (hardcoded from the problem spec):
  x [2048, 2, 1024], gate_w [1024, 8], gate_b [8],
  w1 [8, 1024, 4096], b1 [8, 4096], w2 [8, 4096, 1024], b2 [8, 1024].
"""
import sys
import numpy as np

for _p in ("/opt/trn_rl_repo", "/root/.axon_site/_ro/trn_rl_repo"):
    if _p not in sys.path:
        sys.path.insert(0, _p)

import concourse.bacc as bacc
import concourse.tile as tile
import concourse.mybir as mybir
from concourse import bass2jax, mybir as _mybir

N_EXPERTS = 8
TOP_K = 2
S, B, D, F = 2048, 2, 1024, 4096
P = 128
FB = 512                # F-block size streamed through SBUF
NB = F // FB            # 8 F-blocks
FC = FB // P            # 4 partition-tiles of F per block
DK = D // P             # 8 contraction tiles for stage 1
DO = D // P             # 8 output-partition tiles for stage 2

_f32 = mybir.dt.float32
_bf16 = mybir.dt.bfloat16
_np_bf16 = mybir.dt.np(_bf16)

_NC_CACHE: dict = {}
_C_MAX = 1280           # max capacity per pass (SBUF budget bound)
LAST_DEVICE_NS = -1     # wall-clock of the last device dispatch (incl. transfers)
LAST_SLOTS = None       # slot sizes used by the last kernel() pass
LAST_INMAPS = None      # per-core device inputs of the last kernel() pass


def _c_chunks(C_act):
    """Split C_act (multiple of 4) into k=ceil(C/512) near-equal chunks.

    Chunk sizes are multiples of 4, <=512 (one PSUM bank of fp32) and
    >=256 whenever C_act >= 256 (k chunks only exist when C > 512(k-1)).
    """
    k = -(-C_act // 512)
    base = (C_act // k) // 4 * 4
    sizes = [base] * k
    rem = C_act - base * k
    i = 0
    while rem > 0:
        sizes[i] += 4
        rem -= 4
        i = (i + 1) % k
    out, pos = [], 0
    for s in sizes:
        out.append((pos, s))
        pos += s
    return out


def _build(slots, passes=1, *, skew=True, psum_bufs=8, w_bufs=2,
           h_bufs=2, w1_bufs=None, span=2, dve_relu=True, timing_nodma=False):
    """Trace + compile the per-core SPMD program.

    slots: per-core token-column segments, one expert weight-set each
    (multiples of 4, each >=256 when possible). len(slots)==1 is plain
    expert parallelism; len(slots)==2 balances load by giving each core
    two expert segments (Ca, Cb) chosen by the host dispatcher.
    passes>1 repeats the whole compute (same output) — used only for
    differential timing of the device kernel.
    span: F-blocks accumulated per stage-2 PSUM group (1 or 2); span=2
    halves the stage-2 psum recycle + DVE traffic per pass.
    timing_nodma: reuse block-0 weights for every block (wrong numerics,
    timing-only) to isolate the per-pass DMA cost.
    """
    slots = tuple(slots)
    NS = len(slots)
    Ctot = sum(slots)
    if w1_bufs is None:
        w1_bufs = w_bufs
    if span == 2:
        w_bufs = max(w_bufs, 4)
        h_bufs = max(h_bufs, 4)
    key = (slots, passes, skew, psum_bufs, w_bufs, h_bufs, w1_bufs, span,
           dve_relu, timing_nodma)
    if key in _NC_CACHE:
        return _NC_CACHE[key]
    nc = bacc.Bacc("TRN2", target_bir_lowering=False, debug=False,
                   enable_asserts=False, num_devices=8)
    # host pre-arranged layouts (see _prep_weights): partition dim first
    xg_d = nc.dram_tensor("xg", (P, DK, Ctot), _bf16, kind="ExternalInput").ap()
    w1_d = [nc.dram_tensor(f"w1{s}", (P, NB, DK * FB), _bf16,
                           kind="ExternalInput").ap() for s in range(NS)]
    b1_d = [nc.dram_tensor(f"b1{s}", (F,), _f32,
                           kind="ExternalInput").ap() for s in range(NS)]
    w2_d = [nc.dram_tensor(f"w2{s}", (P, NB * FC, D), _bf16,
                           kind="ExternalInput").ap() for s in range(NS)]
    y_d = nc.dram_tensor("y", (P, DO, Ctot), _f32, kind="ExternalOutput").ap()

    # chunks: (slot, col_start, width) — never cross a slot boundary
    chunks = []
    off = 0
    for s, width in enumerate(slots):
        for cs, csz in _c_chunks(width):
            chunks.append((s, off + cs, csz))
        off += width

    with tile.TileContext(nc) as tc:
        with tc.tile_pool(name="const", bufs=1) as cpool, \
             tc.tile_pool(name="w1p", bufs=w1_bufs) as w1pool, \
             tc.tile_pool(name="w2p", bufs=w_bufs) as w2pool, \
             tc.tile_pool(name="hp", bufs=h_bufs) as hpool, \
             tc.tile_pool(name="ps", bufs=psum_bufs, space="PSUM") as psum:

            # Block-0 head is DMA-bound: interleave xg's dk slices with
            # w1-block0 dk slices so block-0 stage1 (emitted dk-major in
            # waves) can start as soon as the early dk slices land.
            xg_sb = cpool.tile([P, DK, Ctot], _bf16)
            w1_first = w1pool.tile([P, NS, DK, FB], _bf16, tag="w1_t")
            w2_first = w2pool.tile([P, NS, FC, D], _bf16, tag="w2_t")
            for dk in range(DK):
                nc.sync.dma_start(xg_sb[:, dk, :], xg_d[:, dk, :])
                for s in range(NS):
                    nc.sync.dma_start(w1_first[:, s, dk],
                                      w1_d[s][:, 0, dk * FB:(dk + 1) * FB])
                if 4 <= dk < 4 + FC:
                    fk = dk - 4
                    for s in range(NS):
                        nc.sync.dma_start(w2_first[:, s, fk], w2_d[s][:, fk, :])
            b1_sb = cpool.tile([P, NS, F // P], _f32)
            for s in range(NS):
                nc.sync.dma_start(b1_sb[:, s], b1_d[s].rearrange("(o p) -> p o", p=P))
            y_acc = cpool.tile([P, DO, Ctot], _f32)

            def stage1(fb, first_block=False):
                """load w1/w2 block, produce hT = relu(w1.T @ xg + b1)."""
                if first_block or timing_nodma:
                    w1_t, w2_t = w1_first, w2_first
                else:
                    w1_t = w1pool.tile([P, NS, DK, FB], _bf16)
                    w2_t = w2pool.tile([P, NS, FC, D], _bf16)
                    for s in range(NS):
                        nc.sync.dma_start(
                            w1_t[:, s].rearrange("p dk f -> p (dk f)"),
                            w1_d[s][:, fb, :])
                        nc.sync.dma_start(
                            w2_t[:, s].rearrange("p fk d -> p (fk d)"),
                            w2_d[s][:, fb * FC:(fb + 1) * FC, :]
                                .rearrange("p fk d -> p (fk d)"))
                hT = hpool.tile([P, FC, Ctot], _bf16)
                # chunk-major so stage2's chunk-0 groups see all 4 fc relus early
                groups = [(fc, ci) for ci in range(len(chunks)) for fc in range(FC)]

                def relu_group(fc, ci, ps, on_dve=False):
                    sl, cs, csz = chunks[ci]
                    fcol = fb * FC + fc
                    if on_dve:
                        # relu(ps + b1) in one DVE op — where the serial ACT
                        # relu trail would gate psum slot recycling
                        nc.vector.tensor_scalar(
                            hT[:, fc, cs:cs + csz], ps[:, :csz],
                            b1_sb[:, sl, fcol:fcol + 1], 0.0,
                            mybir.AluOpType.add, mybir.AluOpType.max,
                        )
                    else:
                        nc.scalar.activation(
                            hT[:, fc, cs:cs + csz], ps[:, :csz],
                            mybir.ActivationFunctionType.Relu,
                            bias=b1_sb[:, sl, fcol:fcol + 1], scale=1.0,
                        )

                if first_block:
                    # dk-major waves: up to psum_bufs groups accumulate
                    # concurrently so PE consumes each xg[dk] as it lands;
                    # chunk-major so stage2's first groups complete early
                    for ws in range(0, len(groups), psum_bufs):
                        wave = groups[ws:ws + psum_bufs]
                        pss = [psum.tile([P, 512], _f32, name=f"ps_w{ws}_{i}", tag="ps")
                               for i in range(len(wave))]
                        for dk in range(DK):
                            for (fc, ci), ps in zip(wave, pss):
                                sl, cs, csz = chunks[ci]
                                nc.tensor.matmul(
                                    ps[:, :csz],
                                    w1_t[:, sl, dk, fc * P:(fc + 1) * P],
                                    xg_sb[:, dk, cs:cs + csz],
                                    start=(dk == 0), stop=(dk == DK - 1),
                                )
                        for i, ((fc, ci), ps) in enumerate(zip(wave, pss)):
                            relu_group(fc, ci, ps, on_dve=(i % 2 == 0))
                else:
                    for gi, (fc, ci) in enumerate(groups):
                        sl, cs, csz = chunks[ci]
                        ps = psum.tile([P, 512], _f32, tag="ps")
                        for dk in range(DK):
                            nc.tensor.matmul(
                                ps[:, :csz],
                                w1_t[:, sl, dk, fc * P:(fc + 1) * P],
                                xg_sb[:, dk, cs:cs + csz],
                                start=(dk == 0), stop=(dk == DK - 1),
                            )
                        relu_group(fc, ci, ps, on_dve=(dve_relu and gi % 2 == 0))
                return hT, w2_t

            def stage2(pair, first, last):
                """y_acc[d%128, d//128, c] (+)= w2_tile.T @ hT over the pair's
                F-blocks (one PSUM accumulation group each); store on last."""
                nmm = len(pair) * FC
                for do in range(DO):
                    for ci, (sl, cs, csz) in enumerate(chunks):
                        ps2 = psum.tile([P, 512], _f32, tag="ps")
                        mi = 0
                        for hT, w2_t in pair:
                            for fk in range(FC):
                                nc.tensor.matmul(
                                    ps2[:, :csz],
                                    w2_t[:, sl, fk, do * P:(do + 1) * P],
                                    hT[:, fk, cs:cs + csz],
                                    start=(mi == 0), stop=(mi == nmm - 1),
                                )
                                mi += 1
                        ya = y_acc[:, do, cs:cs + csz]
                        if first:
                            nc.vector.tensor_copy(ya, ps2[:, :csz])
                        else:
                            nc.vector.tensor_add(ya, ps2[:, :csz], ya)
                    if last:
                        nc.sync.dma_start(y_d[:, do, :], y_acc[:, do, :])

            n_blocks = passes * NB
            tiles = {}

            def emit_s1(rep):
                tiles[rep] = stage1(rep % NB, first_block=(rep == 0))

            def emit_s2(rep0, nrep):
                pair = [tiles.pop(rep0 + i) for i in range(nrep)]
                fb_last = (rep0 + nrep - 1) % NB
                stage2(pair, rep0 % NB == 0, fb_last == NB - 1)

            if span == 2:
                # pairs (2k, 2k+1); s2(pair k-1) emitted between the next
                # pair's two stage1s so the relu trail hides under stage1.
                emit_s1(0)
                emit_s1(1)
                for k in range(1, n_blocks // 2):
                    emit_s1(2 * k)
                    emit_s2(2 * (k - 1), 2)
                    emit_s1(2 * k + 1)
                emit_s2(n_blocks - 2, 2)
            elif skew and n_blocks >= 2:
                # Block 0 is DMA-bound: emit s2(0) right after s1(0) (PE is
                # waiting on DMA there anyway, the relu bubble is free), then
                # skew by one block so later relus hide under the next s1.
                emit_s1(0)
                emit_s2(0, 1)
                emit_s1(1)
                for rep in range(2, n_blocks):
                    emit_s1(rep)
                    emit_s2(rep - 1, 1)
                emit_s2(n_blocks - 1, 1)
            else:
                for rep in range(n_blocks):
                    emit_s1(rep)
                    emit_s2(rep, 1)
    nc.compile()
    _NC_CACHE[key] = nc
    return nc


class _Runner:
    """Persistent jitted SPMD executor for a compiled Bacc program.

    Mirrors bass2jax.run_bass_via_pjrt but keeps the jitted callable so
    repeat calls skip retracing/recompiling.
    """

    def __init__(self, nc, n_cores):
        import jax
        from jax.sharding import Mesh, PartitionSpec
        from jax.experimental.shard_map import shard_map

        bass2jax.install_neuronx_cc_hook()
        self.nc = nc
        self.n_cores = n_cores
        in_names, out_names, out_avals = [], [], []
        for alloc in nc.m.functions[0].allocations:
            if not isinstance(alloc, _mybir.MemoryLocationSet):
                continue
            name = alloc.memorylocations[0].name
            if alloc.kind == "ExternalInput":
                in_names.append(name)
            elif alloc.kind == "ExternalOutput":
                out_names.append(name)
                out_avals.append(jax.core.ShapedArray(
                    tuple(alloc.tensor_shape), _mybir.dt.np(alloc.dtype)))
        partition_name = nc.partition_id_tensor.name if nc.partition_id_tensor else None
        in_names = [n for n in in_names if n != partition_name]
        all_names = in_names + out_names + ([partition_name] if partition_name else [])
        self.in_names, self.out_names, self.out_avals = in_names, out_names, out_avals
        self._all_names, self._partition_name = all_names, partition_name
        n_params = len(in_names)

        def _body(*args):
            operands = list(args)
            if partition_name is not None:
                operands.append(bass2jax.partition_id_tensor())
            outs = bass2jax._bass_exec_p.bind(
                *operands,
                out_avals=tuple(out_avals),
                in_names=tuple(all_names),
                out_names=tuple(out_names),
                lowering_input_output_aliases=(),
                sim_require_finite=False,
                sim_require_nnan=False,
                nc=nc,
            )
            return tuple(outs)

        devices = jax.devices()[:n_cores]
        mesh = Mesh(np.asarray(devices), ("core",))
        n_outs = len(out_names)
        self._fn = jax.jit(
            shard_map(_body, mesh=mesh,
                      in_specs=(PartitionSpec("core"),) * (n_params + n_outs),
                      out_specs=(PartitionSpec("core"),) * n_outs,
                      check_rep=False),
            donate_argnums=tuple(range(n_params, n_params + n_outs)),
            keep_unused=True,
        )
        self._jax = jax

    def concat_inputs(self, in_maps):
        return [np.concatenate([np.asarray(m[name]) for m in in_maps], axis=0)
                for name in self.in_names]

    def zero_outs(self):
        jnp = self._jax.numpy
        return [jnp.zeros((self.n_cores * a.shape[0], *a.shape[1:]), a.dtype)
                for a in self.out_avals]

    def run_raw(self, concat_in, zouts):
        outs = self._fn(*concat_in, *zouts)
        self._jax.block_until_ready(outs)
        return outs

    def run(self, in_maps):
        outs = self.run_raw(self.concat_inputs(in_maps), self.zero_outs())
        return [
            {name: np.asarray(outs[i]).reshape(self.n_cores, *self.out_avals[i].shape)[c]
             for i, name in enumerate(self.out_names)}
            for c in range(self.n_cores)
        ]


_RUNNER_CACHE: dict = {}


def _runner(slots, passes=1):
    key = (tuple(slots), passes)
    if key not in _RUNNER_CACHE:
        _RUNNER_CACHE[key] = _Runner(_build(slots, passes), N_EXPERTS)
    return _RUNNER_CACHE[key]


def _route(x2d, gate_w, gate_b):
    """Host gate: returns per-token top-2 expert ids and softmax probs (fp32)."""
    logits = x2d.astype(np.float64) @ gate_w.astype(np.float64) + gate_b.astype(np.float64)
    order = np.argsort(-logits, axis=-1, kind="stable")
    top2 = order[:, :TOP_K]                               # [T, 2]
    l = np.take_along_axis(logits, top2, axis=-1)         # [T, 2]
    m = l.max(axis=-1, keepdims=True)
    e = np.exp(l - m)
    p = (e / e.sum(axis=-1, keepdims=True)).astype(np.float32)
    return top2, p


def _prep_weights(w1_e, w2_e):
    """Pre-arrange one expert's weights into the device DMA layouts (bf16).

    w1 [D, F] -> [128(ki), NB, DK*FB] with inner order (ko, fcol):
      block fb's slice [:, fb, :] is one contiguous 8KB/partition DMA.
    w2 [F, D] -> [128(fi), NB*FC, D]: block fb's [:, 4fb:4fb+4, :] ditto.
    """
    w1p = w1_e.reshape(DK, P, NB, FB).transpose(1, 2, 0, 3)      # ki, fb, ko, fcol
    w1p = np.ascontiguousarray(w1p, dtype=_np_bf16).reshape(P, NB, DK * FB)
    w2p = w2_e.reshape(NB * FC, P, D).transpose(1, 0, 2)          # fi, fo, d
    w2p = np.ascontiguousarray(w2p, dtype=_np_bf16)
    return w1p, w2p


def _r4(v):
    return max(256, -(-int(v) // 4) * 4)


def _plan_slots(counts):
    """Choose per-core slot sizes + the expert->slot assignment.

    Returns (slots, percore): percore[c][s] = (expert, lo, hi) — core c's
    slot s holds tokens [lo, hi) of that expert's token list.

    Two-slot plan (when it beats plain expert parallelism): the two
    biggest experts each split across two A slots, the two smallest pair
    up on B slots, the middle four take one A + one B.
    """
    counts = np.asarray(counts)
    order = np.argsort(-counts, kind="stable")
    l = counts[order]
    sym = _r4(l[0])
    Ca = _r4(-(-max(l[0], l[1]) // 2))
    Cb = _r4(max(-(-max(l[6], l[7]) // 2), max(l[2:6].max(), 0) - Ca))
    if Ca + Cb + 16 < sym and Ca + Cb <= 1152:
        top, mid, bot = order[:2], order[2:6], order[6:]
        percore = [[None, None] for _ in range(8)]
        for i, e in enumerate(top):          # cores 2i, 2i+1 A-slots
            n = counts[e]
            k = min(Ca, n)
            percore[2 * i][0] = (e, 0, k)
            percore[2 * i + 1][0] = (e, k, n)
        for i, e in enumerate(mid):          # A of core 4+i, B of core i
            n = counts[e]
            k = min(Ca, n)
            percore[4 + i][0] = (e, 0, k)
            percore[i][1] = (e, k, n)
        for i, e in enumerate(bot):          # cores 4+2i, 5+2i B-slots
            n = counts[e]
            k = min(Cb, n)
            percore[4 + 2 * i][1] = (e, 0, k)
            percore[5 + 2 * i][1] = (e, k, n)
        return (Ca, Cb), percore
    return (sym,), [[(e, 0, counts[e])] for e in range(8)]


def _make_inmaps(xT, idx_lists, wprep, b1, slots, percore):
    """Build per-core device inputs + combine metadata for one pass."""
    Ctot = sum(slots)
    offs = [0]
    for s in slots:
        offs.append(offs[-1] + s)
    in_maps, metas = [], []
    for c in range(N_EXPERTS):
        xg = np.zeros((P, DK, Ctot), dtype=_np_bf16)
        m, mcore = {"xg": xg}, []
        for s in range(len(slots)):
            e, lo, hi = percore[c][s]
            ix = idx_lists[e][0][lo:hi]
            pe = idx_lists[e][1][lo:hi]
            n = len(ix)
            assert n <= slots[s]
            if n:
                # [D, n] -> [128(ki), DK(ko), n]
                xg[:, :, offs[s]:offs[s] + n] = (
                    xT[:, ix].astype(_np_bf16).reshape(DK, P, n).transpose(1, 0, 2))
            m[f"w1{s}"] = wprep[e][0]
            m[f"w2{s}"] = wprep[e][1]
            m[f"b1{s}"] = np.ascontiguousarray(b1[e])
            mcore.append((ix, pe, n, offs[s]))
        in_maps.append(m)
        metas.append(mcore)
    return in_maps, metas


def kernel(x, gate_w, gate_b, w1, b1, w2, b2):
    x = np.asarray(x, dtype=np.float32)
    gate_w = np.asarray(gate_w, dtype=np.float32)
    gate_b = np.asarray(gate_b, dtype=np.float32)
    w1 = np.asarray(w1, dtype=np.float32)
    b1 = np.asarray(b1, dtype=np.float32)
    w2 = np.asarray(w2, dtype=np.float32)
    b2 = np.asarray(b2, dtype=np.float32)

    T = S * B
    x2d = np.ascontiguousarray(x.reshape(T, D))
    top2, p = _route(x2d, gate_w, gate_b)

    # dispatch: token lists per expert
    idx_lists = []
    for e in range(N_EXPERTS):
        sel = np.nonzero(top2 == e)          # (token_idx, slot_idx)
        idx_lists.append((sel[0], p[sel[0], sel[1]]))
    max_n = max(len(ix) for ix, _ in idx_lists)

    # capacity cap (SBUF budget): if wildly imbalanced, run multiple passes
    n_pass = max(1, -(-max_n // _C_MAX))
    per_pass = -(-max_n // n_pass)

    out2d = np.zeros((T, D), dtype=np.float32)
    xT = x2d.T  # [D, T]
    wprep = [_prep_weights(w1[e], w2[e]) for e in range(N_EXPERTS)]
    global LAST_SLOTS, LAST_INMAPS, LAST_DEVICE_NS
    for ps in range(n_pass):
        lists = [(ix[ps * per_pass:(ps + 1) * per_pass],
                  pe[ps * per_pass:(ps + 1) * per_pass]) for ix, pe in idx_lists]
        counts = [len(ix) for ix, _ in lists]
        if n_pass == 1:
            slots, percore = _plan_slots(counts)
        else:   # keep the compile cache small on pathological imbalance
            slots, percore = (_r4(max(counts)),), \
                [[(e, 0, counts[e])] for e in range(N_EXPERTS)]
        runner = _runner(slots)
        in_maps, metas = _make_inmaps(xT, lists, wprep, b1, slots, percore)
        LAST_SLOTS, LAST_INMAPS = slots, in_maps
        import time as _time
        _t0 = _time.time()
        results = runner.run(in_maps)
        LAST_DEVICE_NS = int((_time.time() - _t0) * 1e9)
        Ctot = sum(slots)
        for c in range(N_EXPERTS):
            y = results[c]["y"]                      # [128(di), DO, Ctot]
            y2 = None
            for (ix, pe, n, off) in metas[c]:
                if n:
                    if y2 is None:
                        y2 = y.transpose(1, 0, 2).reshape(D, Ctot)
                    out2d[ix] += (y2[:, off:off + n] * pe).T  # ix unique per expert

    if np.any(b2):
        comb = np.zeros((T, N_EXPERTS), dtype=np.float32)
        np.put_along_axis(comb, top2, p, axis=-1)
        out2d += comb @ b2
    return out2d.reshape(S, B, D)
